# revision 1
# baseline (speedup 1.0000x reference)
"""DMPNN forward on 8 TRN2 NeuronCores.

Sharding: graph-partition nodes 8 ways (block-padded so each 128-graph block
is tile-aligned); AllGather node embeddings h; each core processes the edges
whose dst it owns in a window-sorted stream: indirect-gather h[src], edge
network matmul, modulate (vector, from PSUM), scatter via onehot matmuls into
per-window PSUM accumulators (dynamic (tile,window) pairs shared across
cores), fused with the root transform into h2 AND the (constant-query) first
Set2Set attention step. Remaining Set2Set steps + MLP head run fully local
with SBUF-cached LSTM weights.
"""
import os
import sys
sys.path.insert(0, '/opt/trn_rl_repo')
import numpy as np

NC = 8
N, E, B = 100000, 400000, 4096
MI, F, D = 25, 100, 256
STEPS = 3
BL = B // NC              # 512 graphs/core
GBLK = 4                  # graph blocks of 128
BLKN = 3328               # node slots per block (26 tiles)
NP = GBLK * BLKN          # 13312
NW = NP // 128            # 104 windows
NT_N = NP // 128          # 104 node tiles
TPB = NT_N // GBLK        # 26 node tiles per graph block
NFULL = NC * NP           # 106496
# ETS / NT_E are data-dependent (shared free packing); set by prepare()
ETS = None
NT_E = None


def prepare(inputs):
    x = np.asarray(inputs['x'], np.float32)
    ei = np.asarray(inputs['edge_index']).astype(np.int64)
    ea = np.asarray(inputs['edge_attr'], np.float32)
    batch = np.asarray(inputs['batch']).astype(np.int64)

    gb = np.searchsorted(batch, np.arange(0, B + 1, BL))
    own = np.searchsorted(gb[1:], np.arange(N), side='right')
    cb_start = np.searchsorted(batch, np.arange(NC * GBLK) * 128)
    pp = np.zeros(N, np.int64)
    for cb in range(NC * GBLK):
        lo = cb_start[cb]
        hi = cb_start[cb + 1] if cb + 1 < NC * GBLK else N
        assert hi - lo <= BLKN, (cb, hi - lo)
        pp[lo:hi] = (cb % GBLK) * BLKN + np.arange(hi - lo)

    src, dst = ei[0], ei[1]
    do = own[dst]
    src_g_all = own[src] * NP + pp[src]

    # --- shared free packing: all cores place window w's edges at st[w] ---
    global ETS, NT_E
    core_eids = []
    core_dpp = []
    cnts = np.zeros((NC, NW), np.int64)
    for c in range(NC):
        e_ids = np.nonzero(do == c)[0]
        dpp = pp[dst[e_ids]]
        # sort by window, then src row (ascending for gather locality)
        order = np.lexsort((src_g_all[e_ids], dpp // 128))
        e_ids, dpp = e_ids[order], dpp[order]
        core_eids.append(e_ids)
        core_dpp.append(dpp)
        w_of = dpp // 128
        cnts[c] = np.bincount(w_of, minlength=NW)
    cnt_max = cnts.max(0)
    st = np.concatenate([[0], np.cumsum(cnt_max)])   # window w at [st[w], st[w+1])
    ETS = int(((st[NW] + 127) // 128) * 128)
    NT_E = ETS // 128
    # per-tile window base: window containing the tile's first slot
    tbase_w = (np.searchsorted(st, np.arange(NT_E) * 128, side='right') - 1)
    tbase_w = np.minimum(tbase_w, NW - 1)
    # dynamic (tile, window) pairs, shared across cores
    pairs = []            # list of (t, w, k) in tile order
    nempty = 0
    for w in range(NW):
        t0 = min(st[w] // 128, NT_E - 1)
        if cnt_max[w] == 0:
            # empty (all-padding) window: emit one zero-sel pair so the
            # root transform + fused step-1 still run for it
            nempty += 1
            k = int(np.clip(w - tbase_w[t0], 0, 2))
            pairs.append((t0, w, k))
            continue
        t1 = (st[w] + cnt_max[w] - 1) // 128
        for t in range(t0, t1 + 1):
            k = w - tbase_w[t]
            assert 0 <= k <= 2, (t, w, k)
            pairs.append((t, w, k))
    pairs.sort(key=lambda p: (p[0], p[1]))

    per_core = []
    for c in range(NC):
        lo, hi = gb[c], gb[c + 1]
        xt = np.zeros((MI + 1, NP), np.float32)
        xt[:MI, pp[lo:hi]] = x[lo:hi].T
        xt[MI, :] = 1.0
        gid = np.full(NP, -1.0, np.float32)
        gid[pp[lo:hi]] = (batch[lo:hi] - c * BL).astype(np.float32)
        gid_col = np.ascontiguousarray(gid.reshape(NT_N, 128).T)

        e_ids, dpp = core_eids[c], core_dpp[c]
        w_of = dpp // 128
        rows_e = np.full(ETS, -1, np.int64)
        slot_abs = np.full(ETS, -1.0e6, np.float32)
        for w in range(NW):
            sl = np.searchsorted(w_of, w, 'left')
            sr = np.searchsorted(w_of, w, 'right')
            cnt = sr - sl
            rows_e[st[w]:st[w] + cnt] = e_ids[sl:sr]
            slot_abs[st[w]:st[w] + cnt] = dpp[sl:sr].astype(np.float32)
        valid = rows_e >= 0
        ea_t = np.zeros((F + 1, ETS), np.float32)
        ea_t[:F, valid] = ea[rows_e[valid]].T
        ea_t[F, :] = 1.0
        src_g = np.zeros(ETS, np.int32)
        src_g[valid] = src_g_all[rows_e[valid]].astype(np.int32)
        tbase = tbase_w[np.arange(ETS) // 128]
        slot_rel = (slot_abs - 128.0 * tbase).astype(np.float32)
        slot_rel2 = slot_rel.reshape(NT_E, 128)
        t_arr = np.array([p[0] for p in pairs])
        k_arr = np.array([p[2] for p in pairs])
        sel_all = (slot_rel2[t_arr][:, :, None] ==
                   (128.0 * k_arr[:, None, None] +
                    np.arange(128, dtype=np.float32)[None, None, :]))
        selp = np.ascontiguousarray(
            sel_all.transpose(1, 0, 2).reshape(128, len(pairs) * 128))
        per_core.append(dict(
            xt=xt, gid_col=gid_col, ea_t=ea_t,
            src_g=np.ascontiguousarray(src_g.reshape(NT_E, 128).T),
            slot=np.ascontiguousarray(slot_rel2.T), selp=selp))
    wnames = ['lin0_w', 'lin0_b', 'root_w', 'root_b', 'nn_w', 'nn_b',
              'lstm_wih', 'lstm_whh', 'lstm_b', 'lin1_w', 'lin1_b',
              'lin2_w', 'lin2_b']
    weights = {k: np.ascontiguousarray(np.asarray(inputs[k], np.float32))
               for k in wnames}
    # step-1 Set2Set constants (q_star = 0, hq = 0, c = 0):
    sig = lambda v: 1.0 / (1.0 + np.exp(-v))
    b = weights['lstm_b'].reshape(4 * D)
    gi0, gf0, gg0, go0 = b[:D], b[D:2 * D], b[2 * D:3 * D], b[3 * D:]
    c1 = sig(gi0) * np.tanh(gg0)
    hq1 = sig(go0) * np.tanh(c1)
    weights['hq1_bc'] = np.broadcast_to(hq1, (128, D)).copy()
    weights['c1_bc'] = np.broadcast_to(c1, (128, D)).copy().astype(np.float32)
    # permute LSTM gate columns [i,f,g,o] -> [i,f,o,g] so sigmoid gates are
    # contiguous (batched activation table loads)
    perm = np.concatenate([np.arange(0, 2 * D), np.arange(3 * D, 4 * D),
                           np.arange(2 * D, 3 * D)])
    weights['lstm_wih'] = np.ascontiguousarray(weights['lstm_wih'][:, perm])
    weights['lstm_whh'] = np.ascontiguousarray(weights['lstm_whh'][:, perm])
    weights['lstm_b'] = np.ascontiguousarray(weights['lstm_b'][perm])
    for k, sh in [('lin0_b', D), ('root_b', D), ('nn_b', D),
                  ('lstm_b', 4 * D), ('lin1_b', D), ('lin2_b', 1)]:
        weights[k] = weights[k].reshape(1, sh)
    import ml_dtypes
    bf = ml_dtypes.bfloat16
    weights['lin0_w'] = np.concatenate(
        [weights['lin0_w'], weights['lin0_b'].reshape(1, D)], 0)
    weights['nn_w'] = np.concatenate(
        [weights['nn_w'], weights['nn_b'].reshape(1, D)], 0)
    del weights['lin0_b'], weights['nn_b']
    # SBUF-resident layouts for LSTM / head weights: (k p) d -> p k d
    weights['lstm_wih'] = np.ascontiguousarray(
        weights['lstm_wih'].reshape(4, 128, 4 * D).transpose(1, 0, 2))
    weights['lstm_whh'] = np.ascontiguousarray(
        weights['lstm_whh'].reshape(2, 128, 4 * D).transpose(1, 0, 2))
    weights['lin1_w'] = np.ascontiguousarray(
        weights['lin1_w'].reshape(4, 128, D).transpose(1, 0, 2))
    for k in list(weights):
        if k != 'c1_bc':
            weights[k] = weights[k].astype(bf)
    for pc in per_core:
        pc['xt'] = pc['xt'].astype(bf)
        pc['ea_t'] = pc['ea_t'].astype(bf)
        pc['selp'] = pc['selp'].astype(bf)
    return per_core, weights, pairs


def numpy_device_sim(per_core, weights, pairs):
    W = {k: np.asarray(v, np.float32) for k, v in weights.items()}
    # undo gate permutation for the straightforward sim
    inv = np.concatenate([np.arange(0, 2 * D), np.arange(3 * D, 4 * D),
                          np.arange(2 * D, 3 * D)])
    W['lstm_wih'] = W['lstm_wih'].transpose(1, 0, 2).reshape(2 * D, 4 * D)[:, inv]
    W['lstm_whh'] = W['lstm_whh'].transpose(1, 0, 2).reshape(D, 4 * D)[:, inv]
    W['lstm_b'] = W['lstm_b'].reshape(4 * D)[inv][None, :]
    W['lin1_w'] = W['lin1_w'].transpose(1, 0, 2).reshape(2 * D, D)
    per_core = [dict(pc, xt=np.asarray(pc['xt'], np.float32),
                     ea_t=np.asarray(pc['ea_t'], np.float32))
                for pc in per_core]
    h_all = [np.maximum(pc['xt'].T @ W['lin0_w'], 0.0)
             for pc in per_core]
    h_full = np.concatenate(h_all, 0)
    pairs_by_t = {}
    for (t, w, k) in pairs:
        pairs_by_t.setdefault(t, []).append((w, k))
    outs = []
    for c in range(NC):
        pc = per_core[c]
        ew = pc['ea_t'].T @ W['nn_w']
        srcs = pc['src_g'].T.reshape(ETS)
        msg = h_full[srcs] * ew
        slot = pc['slot'].T.reshape(ETS)
        agg = np.zeros((NP, D), np.float32)
        for t in range(NT_E):
            mt = msg[t * 128:(t + 1) * 128]
            sl = slot[t * 128:(t + 1) * 128]
            for (w, k) in pairs_by_t.get(t, []):
                sel = (sl[:, None] == (128 * k + np.arange(128))[None, :])
                agg[w * 128:(w + 1) * 128] += sel.astype(np.float32).T @ mt
        h2 = np.maximum(h_all[c] @ W['root_w'] + W['root_b'] + agg, 0.0)
        gidc = pc['gid_col'].T.reshape(NP)
        validn = gidc >= 0
        gidi = np.where(validn, gidc, 0).astype(np.int64)
        hq = np.zeros((BL, D), np.float32)
        cc = np.zeros((BL, D), np.float32)
        r = np.zeros((BL, D), np.float32)
        sig = lambda v: 1.0 / (1.0 + np.exp(-v))
        for s in range(STEPS):
            qs = np.concatenate([hq, r], 1)
            gates = qs @ W['lstm_wih'] + hq @ W['lstm_whh'] + W['lstm_b']
            gi, gf, gg, go = np.split(gates, 4, 1)
            cc = sig(gf) * cc + sig(gi) * np.tanh(gg)
            hq = sig(go) * np.tanh(cc)
            e = (h2 * hq[gidi]).sum(1)
            a = np.where(validn, np.exp(e), 0.0)
            z = np.zeros(BL, np.float32)
            np.add.at(z, gidi[validn], a[validn])
            rn = np.zeros((BL, D), np.float32)
            np.add.at(rn, gidi[validn], a[validn, None] * h2[validn])
            r = rn / np.maximum(z, 1e-30)[:, None]
        qs = np.concatenate([hq, r], 1)
        o = np.maximum(qs @ W['lin1_w'] + W['lin1_b'], 0.0) @ W['lin2_w'] + W['lin2_b']
        outs.append(o.reshape(-1))
    return np.concatenate(outs)


def build_nc(pairs):
    from concourse import bass, bacc, mybir
    import concourse.tile as tile
    from concourse.masks import make_identity
    f32, bf16, i32 = mybir.dt.float32, mybir.dt.bfloat16, mybir.dt.int32
    AF = mybir.ActivationFunctionType
    ALU = mybir.AluOpType
    AX = mybir.AxisListType

    pairs_by_t = {}
    for (t, w, k) in pairs:
        pairs_by_t.setdefault(t, []).append((w, k))
    last_tile_of_w = {}
    for (t, w, k) in pairs:
        last_tile_of_w[w] = max(last_tile_of_w.get(w, -1), t)
    fin_after_t = {}
    for w, t in last_tile_of_w.items():
        fin_after_t.setdefault(t, []).append(w)
    for t in fin_after_t:
        fin_after_t[t].sort()

    nc = bacc.Bacc("TRN2", target_bir_lowering=False, debug=False,
                   num_devices=NC)
    P = {}
    def inp(name, shape, dt=f32):
        P[name] = nc.declare_dram_parameter(name, list(shape), dt,
                                            isOutput=False)
    inp('xt', (MI + 1, NP), bf16); inp('gid_col', (128, NT_N))
    inp('ea_t', (F + 1, ETS), bf16); inp('src_g', (128, NT_E), i32)
    inp('selp', (128, len(pairs) * 128), bf16)
    inp('lin0_w', (MI + 1, D), bf16)
    inp('root_w', (D, D), bf16); inp('root_b', (1, D), bf16)
    inp('nn_w', (F + 1, D), bf16)
    inp('lstm_wih', (128, 4, 4 * D), bf16); inp('lstm_whh', (128, 2, 4 * D), bf16)
    inp('lstm_b', (1, 4 * D), bf16)
    inp('lin1_w', (128, 4, D), bf16); inp('lin1_b', (1, D), bf16)
    inp('lin2_w', (D, 1), bf16); inp('lin2_b', (1, 1), bf16)
    inp('hq1_bc', (128, D), bf16); inp('c1_bc', (128, D), f32)
    y = nc.declare_dram_parameter('y', [BL, 1], f32, isOutput=True)

    h_loc = nc.dram_tensor('h_loc', [NP, D], bf16)
    h_t_dram = nc.dram_tensor('h_t_dram', [2, 128, NP], bf16)
    h_full = nc.dram_tensor('h_full', [NFULL, D], bf16, addr_space='Shared')

    with tile.TileContext(nc) as tc:
        with (
            tc.tile_pool(name='wp', bufs=1) as wp,
            tc.tile_pool(name='io', bufs=3) as io,
            tc.tile_pool(name='msgs', bufs=8) as msgs,
            tc.tile_pool(name='s2s', bufs=1) as s2s,
            tc.tile_pool(name='big', bufs=1) as big,
            tc.tile_pool(name='psA', bufs=2, space='PSUM') as psA,
            tc.tile_pool(name='psW', bufs=2, space='PSUM') as psW,
        ):
            def wtile(name, shape, dt=bf16):
                t = wp.tile(list(shape), dt, tag=name, name=name)
                nc.sync.dma_start(out=t[:], in_=P[name][:])
                return t
            lin0_w = wtile('lin0_w', (MI + 1, D))
            nn_w = wtile('nn_w', (F + 1, D))
            root_b = wtile('root_b', (1, D))
            lstm_b = wtile('lstm_b', (1, 4 * D))
            lin1_b = wtile('lin1_b', (1, D))
            lin2_b = wtile('lin2_b', (1, 1))
            wih_sb = wtile('lstm_wih', (128, 4, 4 * D))
            whh_sb = wtile('lstm_whh', (128, 2, 4 * D))
            lin1_sb = wtile('lin1_w', (128, 4, D))
            hq1_bc = wtile('hq1_bc', (128, D))
            c1_bc = wtile('c1_bc', (128, D), f32)
            root_w = []
            for kc in range(2):
                t = wp.tile([128, D], bf16, tag=f'rootw{kc}', name=f'rootw{kc}')
                nc.sync.dma_start(out=t[:], in_=P['root_w'][kc * 128:(kc + 1) * 128, :])
                root_w.append(t)
            lin2_w = []
            for kc in range(2):
                t = wp.tile([128, 1], bf16, tag=f'lin2w{kc}', name=f'lin2w{kc}')
                nc.sync.dma_start(out=t[:], in_=P['lin2_w'][kc * 128:(kc + 1) * 128, :])
                lin2_w.append(t)
            ones = wp.tile([1, 512], bf16, tag='ones', name='ones')
            nc.vector.memset(ones[:], 1.0)
            iota_i = wp.tile([128, 128], i32, tag='iota_i', name='iota_i')
            nc.gpsimd.iota(iota_i[:], pattern=[[1, 128]], base=0,
                           channel_multiplier=0)
            iotas = []
            for k, base in enumerate((0.0, 128.0, 256.0)):
                it = wp.tile([128, 128], f32, tag=f'iota{k}', name=f'iota{k}')
                nc.scalar.activation(out=it[:], in_=iota_i[:], func=AF.Copy,
                                     bias=base)
                iotas.append(it)
            ic_i = wp.tile([128, 1], i32, tag='iotac_i', name='iotac_i')
            nc.gpsimd.iota(ic_i[:], pattern=[[1, 1]], base=0,
                           channel_multiplier=1)
            iota_col = wp.tile([128, 1], f32, tag='iotac', name='iotac')
            nc.vector.tensor_copy(out=iota_col[:], in_=ic_i[:])
            ident = wp.tile([128, 128], bf16, tag='ident', name='ident')
            make_identity(nc, ident[:])
            identf = wp.tile([128, 128], f32, tag='identf', name='identf')
            make_identity(nc, identf[:])
            gidc_sb = wp.tile([128, NT_N], f32, tag='gidc', name='gidc')
            nc.sync.dma_start(out=gidc_sb[:], in_=P['gid_col'][:])
            srcg_sb = big.tile([128, NT_E], i32, tag='srcg', name='srcg')
            nc.sync.dma_start(out=srcg_sb[:], in_=P['src_g'][:])

            # ---- phase 0 pass 1: h node-major -> h_loc (feeds the AllGather)
            ctx0 = nc.named_scope('phase0'); ctx0.__enter__()
            for ch in range(NP // 512):
                xc = io.tile([MI + 1, 512], bf16, tag='xtc', name='xtc', bufs=4)
                nc.scalar.dma_start(out=xc[:], in_=P['xt'][:, ch * 512:(ch + 1) * 512])
                hts = io.tile([128, 4, D], bf16, tag='h0out', name='h0out')
                for j in range(4):
                    ps = psA.tile([128, D], f32, space='PSUM',
                                  tag=('mm' if j % 2 else 'mms'), name='mm')
                    nc.tensor.matmul(out=ps[:], lhsT=(xc[:, j * 128:(j + 1) * 128]),
                                     rhs=(lin0_w[:]), start=True, stop=True)
                    nc.vector.tensor_scalar(out=hts[:, j, :], in0=ps[:],
                                            scalar1=0.0, scalar2=None,
                                            op0=ALU.max)
                dst = h_loc[ch * 512:(ch + 1) * 512, :].rearrange(
                    "(k p) d -> p k d", p=128)
                nc.sync.dma_start(out=dst, in_=hts[:])
            ctx0.__exit__(None, None, None)

            # ---- allgather trigger (waits only on h_loc writes)
            ctx1 = nc.named_scope('allgather'); ctx1.__enter__()
            nc.gpsimd.collective_compute(
                'AllGather', ALU.bypass, replica_groups=[list(range(NC))],
                ins=[h_loc[:]], outs=[h_full[:]])
            ctx1.__exit__(None, None, None)

            # ---- sel grids (pure SBUF compute; fills the collective window)
            ctxs = nc.named_scope('selgrids'); ctxs.__enter__()
            selg_all = big.tile([128, NT_N * 128], bf16, tag='selga',
                                name='selga')
            selt_all = big.tile([128, NT_N * 128], bf16, tag='selta',
                                name='selta')
            for t in range(NT_N):
                b = t // TPB
                gT_ps = psA.tile([128, 128], f32, space='PSUM', tag='mm',
                                 name='mm')
                nc.tensor.transpose(
                    out=gT_ps[:],
                    in_=gidc_sb[:, t:t + 1].to_broadcast([128, 128]),
                    identity=identf[:])
                gT = msgs.tile([128, 128], f32, tag='gT', name='gT', bufs=4)
                nc.scalar.activation(out=gT[:], in_=gT_ps[:], func=AF.Copy,
                                     bias=-128.0 * b)
                nc.vector.tensor_tensor(
                    out=selg_all[:, t * 128:(t + 1) * 128],
                    in0=iota_col[:].to_broadcast([128, 128]),
                    in1=gT[:], op=ALU.is_equal)
                gcol = msgs.tile([128, 1], f32, tag='gcol', name='gcol')
                nc.vector.tensor_scalar(
                    out=gcol[:], in0=gidc_sb[:, t:t + 1],
                    scalar1=-128.0 * b, scalar2=None, op0=ALU.add)
                nc.vector.tensor_tensor(
                    out=selt_all[:, t * 128:(t + 1) * 128],
                    in0=gcol[:].to_broadcast([128, 128]),
                    in1=iotas[0][:], op=ALU.is_equal)
            h2 = big.tile([128, NT_N * (D + 1)], bf16, tag='h2', name='h2')
            for w in range(NW):
                nc.vector.memset(h2[:, w * (D + 1) + D:(w + 1) * (D + 1)], 1.0)
            ctxs.__exit__(None, None, None)

            # ---- phase 0 pass 2: h dim-major -> h_t_dram (DMA-heavy; will
            # crawl during the collective, completes shortly after)
            ctx0b = nc.named_scope('phase0b'); ctx0b.__enter__()
            for ch in range(NP // 512):
                xc = io.tile([MI + 1, 512], bf16, tag='xtc', name='xtc', bufs=4)
                nc.sync.dma_start(out=xc[:], in_=P['xt'][:, ch * 512:(ch + 1) * 512])
                for half in range(2):
                    ps = psA.tile([128, 512], f32, space='PSUM',
                                  tag=('mm' if half else 'mms'), name='mm')
                    nc.tensor.matmul(
                        out=ps[:], lhsT=(lin0_w[:, half * 128:(half + 1) * 128]),
                        rhs=(xc[:]), start=True, stop=True)
                    htt = io.tile([128, 512], bf16, tag='h0outt', name='h0outt')
                    nc.vector.tensor_scalar(out=htt[:], in0=ps[:], scalar1=0.0,
                                            scalar2=None, op0=ALU.max)
                    nc.sync.dma_start(
                        out=h_t_dram[half, :, ch * 512:(ch + 1) * 512], in_=htt[:])
            ctx0b.__exit__(None, None, None)

            # ---- phase 1: messages -> windowed scatter -> h2 (+ step-1
            # attention with host-computed constant query hq1)
            ctx2 = nc.named_scope('phase1'); ctx2.__enter__()
            wpsums = {}
            ea_chunk = [None]
            ht_chunk = [None]
            rps1 = [None]
            sel_chunk = [None]
            pair_idx = [0]

            def finalize(wd):
                ps = wpsums.pop(wd)
                if wd % 4 == 0:
                    ht_chunk[0] = io.tile([128, 2, 512], bf16, tag='htc',
                                          name='htc', bufs=3)
                    for half in range(2):
                        nc.scalar.dma_start(
                            out=ht_chunk[0][:, half, :],
                            in_=h_t_dram[half, :, wd * 128:wd * 128 + 512])
                for half in range(2):
                    nc.tensor.matmul(
                        out=ps[:],
                        lhsT=(ht_chunk[0][:, half,
                              (wd % 4) * 128:(wd % 4 + 1) * 128]),
                        rhs=(root_w[half][:]), start=False, stop=False)
                nc.tensor.matmul(out=ps[:], lhsT=(ones[:, :128]),
                                 rhs=(root_b[:]), start=False, stop=True)
                nc.scalar.activation(
                    out=h2[:, wd * (D + 1):wd * (D + 1) + D],
                    in_=ps[:], func=AF.Relu)
                # fused step-1 attention for this window (constant query hq1)
                b = wd // TPB
                j = wd % TPB
                prod = msgs.tile([128, D], bf16, tag='prod', name='prod',
                                 bufs=4)
                nc.vector.tensor_tensor(
                    out=prod[:], in0=h2[:, wd * (D + 1):wd * (D + 1) + D],
                    in1=hq1_bc[:], op=ALU.mult)
                ecol = msgs.tile([128, 1], f32, tag='ecol', name='ecol')
                nc.vector.tensor_reduce(out=ecol[:], in_=prod[:], axis=AX.X,
                                        op=ALU.add)
                acol = msgs.tile([128, 1], f32, tag='acol', name='acol')
                nc.scalar.activation(out=acol[:], in_=ecol[:], func=AF.Exp)
                sela = msgs.tile([128, 128], bf16, tag='sela', name='sela',
                                 bufs=4)
                nc.vector.tensor_tensor(
                    out=sela[:], in0=selt_all[:, wd * 128:(wd + 1) * 128],
                    in1=acol[:].to_broadcast([128, 128]), op=ALU.mult)
                if j == 0:
                    rps1[0] = psW.tile([128, D + 1], f32, space='PSUM',
                                       tag='rps1', name='rps1', bufs=1)
                nc.tensor.matmul(
                    out=rps1[0][:], lhsT=(sela[:]),
                    rhs=(h2[:, wd * (D + 1):(wd + 1) * (D + 1)]),
                    start=(j == 0), stop=(j == TPB - 1))
                if j == TPB - 1:
                    zc = io.tile([128, 1], f32, tag='zc', name='zc')
                    nc.vector.tensor_scalar(out=zc[:], in0=rps1[0][:, D:D + 1],
                                            scalar1=1e-30, scalar2=None,
                                            op0=ALU.max)
                    zr = io.tile([128, 1], f32, tag='zr', name='zr')
                    nc.vector.reciprocal(out=zr[:], in_=zc[:])
                    nc.vector.tensor_scalar(out=rr[b][:], in0=rps1[0][:, :D],
                                            scalar1=zr[:, :1], scalar2=None,
                                            op0=ALU.mult)

            # set2set state tiles (rr filled by fused step 1)
            hq = [s2s.tile([128, D], bf16, tag=f'hq{b}', name=f'hq{b}')
                  for b in range(GBLK)]
            cst = [s2s.tile([128, D], f32, tag=f'c{b}', name=f'c{b}')
                   for b in range(GBLK)]
            rr = [s2s.tile([128, D], bf16, tag=f'r{b}', name=f'r{b}')
                  for b in range(GBLK)]
            for b in range(GBLK):
                nc.vector.tensor_copy(out=hq[b][:], in_=hq1_bc[:])
                nc.vector.tensor_copy(out=cst[b][:], in_=c1_bc[:])

            for t in range(NT_E):
                if t % 8 == 0:
                    cw = min(1024, ETS - t * 128)
                    ea_chunk[0] = io.tile([F + 1, 1024], bf16, tag='ea',
                                          name='ea', bufs=3)
                    nc.scalar.dma_start(
                        out=ea_chunk[0][:, :cw],
                        in_=P['ea_t'][:, t * 128:t * 128 + cw])
                lhs = ea_chunk[0][:, (t % 8) * 128:(t % 8 + 1) * 128]
                ewp = psA.tile([128, D], f32, space='PSUM', tag='ew',
                               name='ew', bufs=1)
                nc.tensor.matmul(out=ewp[:], lhsT=(lhs), rhs=(nn_w[:]),
                                 start=True, stop=True)
                hs = msgs.tile([128, D], bf16, tag='hs', name='hs', bufs=12)
                nc.gpsimd.indirect_dma_start(
                    out=hs[:], out_offset=None, in_=h_full[:],
                    in_offset=bass.IndirectOffsetOnAxis(
                        ap=srcg_sb[:, t:t + 1], axis=0))
                msg = msgs.tile([128, D], bf16, tag='msg', name='msg', bufs=12)
                nc.vector.tensor_tensor(out=msg[:], in0=hs[:], in1=ewp[:],
                                        op=ALU.mult)
                for (w, k) in pairs_by_t.get(t, []):
                    first = w not in wpsums
                    if first:
                        wpsums[w] = psW.tile([128, D], f32, space='PSUM',
                                             tag='aggw', name='aggw')
                    pi = pair_idx[0]
                    pair_idx[0] += 1
                    if pi % 8 == 0:
                        pw = min(1024, (len(pairs) - pi) * 128)
                        sel_chunk[0] = io.tile([128, 1024], bf16, tag='selc',
                                               name='selc', bufs=3)
                        nc.scalar.dma_start(
                            out=sel_chunk[0][:, :pw],
                            in_=P['selp'][:, pi * 128:pi * 128 + pw])
                    sel = sel_chunk[0][:, (pi % 8) * 128:(pi % 8 + 1) * 128]
                    nc.tensor.matmul(out=wpsums[w][:], lhsT=(sel),
                                     rhs=(msg[:]), start=first, stop=False)
                for wd in fin_after_t.get(t, []):
                    finalize(wd)
            assert not wpsums

            ctx2.__exit__(None, None, None)
            ctx3 = nc.named_scope('set2set'); ctx3.__enter__()
            # ---- set2set steps 2..STEPS
            e_grid = s2s.tile([128, NT_N], f32, tag='egrid', name='egrid')
            a_grid = s2s.tile([128, NT_N], f32, tag='agrid', name='agrid')

            def transpose128(src_ap):
                ps = psW.tile([128, 128], bf16, space='PSUM', tag='aggw',
                              name='tps')
                nc.tensor.transpose(out=ps[:], in_=src_ap, identity=ident[:])
                out = io.tile([128, 128], bf16, tag='tout', name='tout', bufs=6)
                nc.vector.tensor_copy(out=out[:], in_=ps[:])
                return out

            for step in range(1, STEPS):
                # LSTM for all blocks, activations batched by function
                gps = {}
                for grp in range(2):
                    blks = [grp * 2, grp * 2 + 1]
                    for b in blks:
                        qsT = [transpose128(hq[b][:, h * 128:(h + 1) * 128])
                               for h in range(2)]
                        qsT += [transpose128(rr[b][:, h * 128:(h + 1) * 128])
                                for h in range(2)]
                        for nh in range(2):
                            g = psA.tile([128, 512], f32, space='PSUM',
                                         tag=('mm' if nh == 0 else 'mms'),
                                         name='gps')
                            gps[(b, nh)] = g
                            for kc in range(4):
                                nc.tensor.matmul(
                                    out=g[:], lhsT=(qsT[kc][:]),
                                    rhs=(wih_sb[:, kc,
                                         nh * 512:(nh + 1) * 512]),
                                    start=(kc == 0), stop=False)
                            for kc in range(2):
                                nc.tensor.matmul(
                                    out=g[:], lhsT=(qsT[kc][:]),
                                    rhs=(whh_sb[:, kc,
                                         nh * 512:(nh + 1) * 512]),
                                    start=False, stop=False)
                            nc.tensor.matmul(
                                out=g[:], lhsT=(ones[:, :128]),
                                rhs=(lstm_b[:, nh * 512:(nh + 1) * 512]),
                                start=False, stop=True)
                    # sigmoid batch: gi|gf (gps0 full), go (gps1 first half)
                    sg = {}
                    for b in blks:
                        s0 = io.tile([128, 512], f32, tag='sg0', name='sg0',
                                     bufs=2)
                        nc.scalar.activation(out=s0[:], in_=gps[(b, 0)][:],
                                             func=AF.Sigmoid)
                        s1 = io.tile([128, D], f32, tag='sg1', name='sg1',
                                     bufs=2)
                        nc.scalar.activation(out=s1[:], in_=gps[(b, 1)][:, :D],
                                             func=AF.Sigmoid)
                        sg[b] = (s0, s1)
                    # tanh batch: gg (gps1 second half)
                    tg = {}
                    for b in blks:
                        t1 = io.tile([128, D], f32, tag='tg', name='tg',
                                     bufs=2)
                        nc.scalar.activation(out=t1[:], in_=gps[(b, 1)][:, D:],
                                             func=AF.Tanh)
                        tg[b] = t1
                    # elementwise LSTM state update
                    for b in blks:
                        gi = sg[b][0][:, :D]
                        gf = sg[b][0][:, D:]
                        go = sg[b][1]
                        gg = tg[b]
                        nc.gpsimd.tensor_tensor(out=cst[b][:], in0=gf,
                                                in1=cst[b][:], op=ALU.mult)
                        ig = io.tile([128, D], f32, tag='ig', name='ig',
                                     bufs=2)
                        nc.gpsimd.tensor_tensor(out=ig[:], in0=gi, in1=gg[:],
                                                op=ALU.mult)
                        nc.gpsimd.tensor_tensor(out=cst[b][:], in0=cst[b][:],
                                                in1=ig[:], op=ALU.add)
                        tct = io.tile([128, D], f32, tag='tanc', name='tanc',
                                      bufs=2)
                        nc.scalar.activation(out=tct[:], in_=cst[b][:],
                                             func=AF.Tanh)
                        nc.gpsimd.tensor_tensor(out=hq[b][:], in0=go[:],
                                                in1=tct[:], op=ALU.mult)
                # attention pass A: scores e per tile, batched exp per block;
                # pass B: ah2 scaling with static selt + matmul reduction
                for b in range(GBLK):
                    for j in range(TPB):
                        t = b * TPB + j
                        hqn = psW.tile([128, D], f32, space='PSUM', tag='aggw',
                                       name='aggw')
                        nc.tensor.matmul(
                            out=hqn[:],
                            lhsT=(selg_all[:, t * 128:(t + 1) * 128]),
                            rhs=(hq[b][:]), start=True, stop=True)
                        prod = msgs.tile([128, D], bf16, tag='prod',
                                         name='prod', bufs=4)
                        nc.vector.tensor_tensor(
                            out=prod[:],
                            in0=h2[:, t * (D + 1):t * (D + 1) + D],
                            in1=hqn[:], op=ALU.mult)
                        scr = msgs.tile([128, D], bf16, tag='scr',
                                        name='scr', bufs=4)
                        nc.scalar.activation(out=scr[:], in_=prod[:],
                                             func=AF.Copy,
                                             accum_out=e_grid[:, t:t + 1])
                    nc.scalar.activation(
                        out=a_grid[:, b * TPB:(b + 1) * TPB],
                        in_=e_grid[:, b * TPB:(b + 1) * TPB], func=AF.Exp)
                    for j in range(TPB):
                        t = b * TPB + j
                        if j == 0:
                            rps = psA.tile([128, D + 1], f32, space='PSUM',
                                           tag='mms', name='mms')
                        sela = msgs.tile([128, 128], bf16, tag='sela',
                                         name='sela', bufs=4)
                        nc.vector.tensor_tensor(
                            out=sela[:],
                            in0=selt_all[:, t * 128:(t + 1) * 128],
                            in1=a_grid[:, t:t + 1].to_broadcast([128, 128]),
                            op=ALU.mult)
                        nc.tensor.matmul(
                            out=rps[:], lhsT=(sela[:]),
                            rhs=(h2[:, t * (D + 1):(t + 1) * (D + 1)]),
                            start=(j == 0), stop=(j == TPB - 1))
                        if j == TPB - 1:
                            zc = io.tile([128, 1], f32, tag='zc', name='zc')
                            nc.vector.tensor_scalar(out=zc[:],
                                                    in0=rps[:, D:D + 1],
                                                    scalar1=1e-30,
                                                    scalar2=None, op0=ALU.max)
                            zr = io.tile([128, 1], f32, tag='zr', name='zr')
                            nc.vector.reciprocal(out=zr[:], in_=zc[:])
                            nc.vector.tensor_scalar(out=rr[b][:],
                                                    in0=rps[:, :D],
                                                    scalar1=zr[:, :1],
                                                    scalar2=None,
                                                    op0=ALU.mult)

            ctx3.__exit__(None, None, None)
            ctx4 = nc.named_scope('head'); ctx4.__enter__()
            # ---- head
            for b in range(GBLK):
                qsT = [transpose128(hq[b][:, h * 128:(h + 1) * 128])
                       for h in range(2)]
                qsT += [transpose128(rr[b][:, h * 128:(h + 1) * 128])
                        for h in range(2)]
                o1ps = psA.tile([128, D], f32, space='PSUM', tag='mm', name='mm')
                for kc in range(4):
                    nc.tensor.matmul(out=o1ps[:], lhsT=(qsT[kc][:]),
                                     rhs=(lin1_sb[:, kc, :]),
                                     start=(kc == 0), stop=False)
                nc.tensor.matmul(out=o1ps[:], lhsT=(ones[:, :128]),
                                 rhs=(lin1_b[:]), start=False, stop=True)
                o1 = io.tile([128, D], bf16, tag='o1s', name='o1s')
                nc.scalar.activation(out=o1[:], in_=o1ps[:], func=AF.Relu)
                o1T = [transpose128(o1[:, h * 128:(h + 1) * 128])
                       for h in range(2)]
                ops_ = psA.tile([128, 1], f32, space='PSUM', tag='mm', name='mm')
                for h in range(2):
                    nc.tensor.matmul(out=ops_[:], lhsT=(o1T[h][:]),
                                     rhs=(lin2_w[h][:]),
                                     start=(h == 0), stop=False)
                nc.tensor.matmul(out=ops_[:], lhsT=(ones[:, :128]),
                                 rhs=(lin2_b[:]), start=False, stop=True)
                ot = io.tile([128, 1], f32, tag='oy', name='oy')
                nc.vector.tensor_copy(out=ot[:], in_=ops_[:])
                nc.sync.dma_start(out=y[b * 128:(b + 1) * 128, :], in_=ot[:])
    ctx4.__exit__(None, None, None)
    nc.finalize()
    return nc


_CACHED = {}


def kernel(**inputs):
    from concourse.bass_utils import run_bass_kernel_spmd
    per_core, weights, pairs = prepare(inputs)
    if 'nc' not in _CACHED:
        _CACHED['nc'] = build_nc(pairs)
    nc = _CACHED['nc']
    in_maps = []
    for c in range(NC):
        m = dict(per_core[c])
        m.pop('slot', None)   # device uses host-precomputed selp instead
        m.update(weights)
        in_maps.append(m)
    res = run_bass_kernel_spmd(nc, in_maps, list(range(NC)),
                               trace=bool(int(os.environ.get('DMPNN_TRACE', '0'))))
    _CACHED['last_exec_ns'] = res.exec_time_ns
    _CACHED['res'] = res
    out = np.concatenate([res.results[c]['y'].reshape(-1) for c in range(NC)])
    return out.astype(np.float32)

